# revision 29
# baseline (speedup 1.0000x reference)
"""AdaptiveRankingLoss distributed Bass kernel for 8 TRN2 NeuronCores.

Math
----
reference loss = sum_{i<j, t_i != t_j} w_ij * relu(margin_ij - sign(t_i - t_j)*(p_i - p_j))
                 / count,
  margin = 0.1 * clip(|t_i - t_j|, 0.1, 1.0),  w = 1/(1 + u_i + u_j).

The summand is symmetric under i<->j, and splitting by the sign of
a = t_j - t_i gives an exactly equivalent full-matrix form with no sign(),
no abs() and no triangular mask:

    numerator = sum_{all i,j} [a_ij > 0] * w_ij * relu(clip(0.1*a_ij, .01, .1) - (p_j - p_i))

Ties (a == 0, including the diagonal) contribute exactly 0 via the
indicator, and `count` is computed exactly on the host from duplicate
analysis of t.

Device mapping (per core: 1024 rows x 8192 cols of the pair matrix)
------------------------------------------------------------------
* one custom 8-stage DVE op produces v = [a>0]*relu(clip(0.1a,.01,.1)-b)
  per element (fp32 internal, bf16 out), streaming the broadcast column
  vectors with the row values as per-partition scalars.
* the weight w = 1/(1+u_i+u_j) is applied through a degree-6 bilinear
  polynomial 1/(2+z) ~ p(z), z = x_i + x_j, x = u - 0.5:
      w_ij ~ sum_n Phi_n(x_i) * x_j^n
  so  sum_ij v_ij w_ij = sum_{n,j} X[n,j] * Psi[n,j]  with
      X[n,j] = sum_i Phi_n(x_i) v_ij   (TensorEngine matmul, PSUM accum)
      Psi[n,j] = x_j^n.
* drains: X is staged out of PSUM by the (otherwise idle) scalar engine,
  partition-reshaped to [112, 256] by DMA, and contracted against Psi by a
  fused custom multiply-reduce; the tail piece reads PSUM directly. The
  host sums the per-core accumulators and divides by the exact pair count.

Host-side marshalling: inputs are sorted by target (the loss is
permutation-invariant) with rows strided across cores, so the [a>0]
indicator becomes triangular and whole column ranges are provably zero
and skipped (bit-exact). Columns are pre-scaled by 0.1 and cast to fp16.
"""

import numpy as np

import concourse.bass as bass
import concourse.bacc as bacc
import concourse.mybir as mybir
import concourse.tile as tile
from concourse.bass_utils import run_bass_kernel_spmd
from concourse import dve_ops
from concourse.dve_spec import (
    Spec,
    Src0,
    Src1,
    C0,
    C1,
    C2,
    Zero,
    relu,
    maxx,
    minn,
    lower,
    _has_src1,
)
from concourse.dve_uop import DveOpSpec

F32 = mybir.dt.float32
BF16 = mybir.dt.bfloat16

N = 8192          # problem size (hardcoded per spec)
NCORES = 8
P = 128           # SBUF partitions
R = N // NCORES   # rows per core (1024)
RT = R // P       # row tiles per core (8)
FC = 1024         # column chunk
NCH = N // FC     # chunks (8)
DEG = 6           # weight polynomial degree
K = DEG + 1
MMF = 512         # matmul free-dim tile

# Inputs are sorted by target on the host and rows are strided across cores
# (core c gets sorted rows c, c+8, ...). Row-tile r of any core then covers
# sorted positions >= 1024*r, so column chunks c < r satisfy t_j <= t_i
# everywhere -> the [a>0] indicator is identically 0 and the chunk is skipped
# for that tile. Bit-exact with the unskipped computation.


# --------------------------------------------------------------------------
# custom DVE op: v = [Src0 - C0 > 0] * relu(clip(Src0 - C0, C2^2, C2) - (Src1 - C1))
# Src0 = 0.1*t_col, C0 = 0.1*t_row, Src1 = p_col, C1 = p_row, C2 = 0.1.
# --------------------------------------------------------------------------
_ARL_NAME = "ARL_MAIN_V1"


def _arl_reference(in0, in1, s0, s1, imm2):
    a = in0 - s0
    m = np.clip(a, np.float32(imm2) * np.float32(imm2), imm2)
    return (a > 0).astype(np.float32) * np.maximum(m - (in1 - s1), 0.0)


def _register_arl_op():
    for op in dve_ops.OPS:
        if op.name == _ARL_NAME:
            return op
    a = Src0 - C0
    m = minn(maxx(a, C2 * C2), C2)
    h = relu(m - (Src1 - C1))
    spec = Spec(body=(a > Zero) * h, reference=_arl_reference)
    row = dve_ops._CUSTOM_DVE_ROW_BASE + len(dve_ops.OPS)
    assert row < 0x20, "custom-DVE row overflow"
    dve_ops._SUB_OPCODE_FOR_NAME[_ARL_NAME] = row
    shas = {}
    for ver in ("v3", "v4"):
        try:
            uops = lower(spec, ver=ver)
            shas[ver] = DveOpSpec(
                name=_ARL_NAME, opcode=row, uops=uops, rd1_en=_has_src1(spec)
            ).sha(ver)
        except Exception:
            pass
    op = dve_ops.DveOp(_ARL_NAME, spec, subdim=False, uops_sha=shas)
    dve_ops.OPS.append(op)
    dve_ops.CUSTOM_DVE_SPECS[_ARL_NAME] = spec
    return op


ARL_MAIN = _register_arl_op()


# --------------------------------------------------------------------------
# degree-6 bilinear split of w = 1/(1+u_i+u_j) = 1/(2 + x_i + x_j), x = u-.5
# --------------------------------------------------------------------------
def _acoef_matrix() -> np.ndarray:
    from numpy.polynomial import chebyshev as _C
    from math import comb

    nodes = np.cos((2 * np.arange(DEG + 1) + 1) / (2 * (DEG + 1)) * np.pi)
    ch = _C.chebfit(nodes, 1.0 / (2.0 + nodes), DEG)
    c = _C.cheb2poly(ch)  # power-basis coeffs of p(z) ~ 1/(2+z) on [-1,1]
    A = np.zeros((K, K), np.float64)
    for mm in range(K):
        for nn in range(K):
            if mm + nn <= DEG:
                A[mm, nn] = c[mm + nn] * comb(mm + nn, mm)
    return A.astype(np.float32)


_ACOEF = _acoef_matrix()


# --------------------------------------------------------------------------
# device graph builder
# --------------------------------------------------------------------------
def _build_nc():
    from contextlib import ExitStack

    F16 = mybir.dt.float16
    HW = N // 2  # column half-width (4096)

    nc = bacc.Bacc(None, target_bir_lowering=False, debug=False)

    t01_ext = nc.declare_dram_parameter("t01col", [N], F16, isOutput=False)
    p_ext = nc.declare_dram_parameter("pcol", [N], F16, isOutput=False)
    u_ext = nc.declare_dram_parameter("ucol", [N], F32, isOutput=False)
    rows_ext = nc.declare_dram_parameter("rows3", [3, R], F32, isOutput=False)
    a_ext = nc.declare_dram_parameter("acoef", [K, K], F32, isOutput=False)
    out_ext = nc.declare_dram_parameter("out", [336], F32, isOutput=True)

    with tile.TileContext(nc) as tc, ExitStack() as ctx:
        constp = ctx.enter_context(tc.tile_pool(name="const", bufs=1))
        colp = ctx.enter_context(tc.tile_pool(name="cols", bufs=1))
        vp = ctx.enter_context(tc.tile_pool(name="v", bufs=3))
        pp = ctx.enter_context(tc.tile_pool(name="psum", bufs=1, space="PSUM"))
        sp = ctx.enter_context(tc.tile_pool(name="small", bufs=1))
        dramp = ctx.enter_context(tc.tile_pool(name="dram", bufs=1, space="DRAM"))

        # ---- small prep DMAs; one fused row-scalar load + u/a coefs ----
        rows_sb = constp.tile([P, 3, RT], F32)
        rows_src = bass.AP(
            tensor=rows_ext, offset=0, ap=[[1, P], [R, 3], [P, RT]]
        )
        nc.sync.dma_start(rows_sb[:], rows_src)
        t01row_sb = rows_sb[:, 0, :]
        prow_sb = rows_sb[:, 1, :]
        urow_sb = rows_sb[:, 2, :]
        abuf = constp.tile([P, K, K], F32)
        a_src = bass.AP(tensor=a_ext, offset=0, ap=[[0, P], [K, K], [1, K]])
        nc.sync.dma_start(abuf[:], a_src)
        FB = N // P  # 64
        u64 = sp.tile([P, FB], F32)
        nc.sync.dma_start(u64[:], u_ext[:].rearrange("(p f) -> p f", p=P))
        # ---- full-width fp16 column tiles; upper half (processed first)
        # DMA'd first ----
        t01_sb = colp.tile([P, N], F16)
        p_sb = colp.tile([P, N], F16)
        for lo in (7168, 6144, 5120, 4096, 2048, 0):
            w = 1024 if lo >= HW else 2048
            nc.sync.dma_start(
                t01_sb[:, lo : lo + w],
                bass.AP(tensor=t01_ext, offset=lo, ap=[[0, P], [1, w]]),
            )
            nc.sync.dma_start(
                p_sb[:, lo : lo + w],
                bass.AP(tensor=p_ext, offset=lo, ap=[[0, P], [1, w]]),
            )


        # ---- pairwise compute (see module docstring). The two smallest
        # upper-half row-tiles are emitted before the Phi/Psi prep so the
        # DVE starts the bulk work as soon as the tail columns land; prep
        # then fills the stream while the remaining columns load. ----
        Xh = {}

        def emit_group(half, cbase, tiles, r):
            v = emit_main(half, cbase, r)
            emit_matmuls(half, cbase, tiles, r, v)

        def emit_main(half, cbase, r):
            c0 = max(cbase, r * 1024)
            w = cbase + HW - c0
            v = vp.tile([P, HW], BF16, tag="v", name=f"v{half}_{r}")
            nc.vector._custom_dve(
                ARL_MAIN,
                out=v[:, :w],
                in0=t01_sb[:, c0 : cbase + HW],
                in1=p_sb[:, c0 : cbase + HW],
                s0=t01row_sb[:, r : r + 1],
                s1=prow_sb[:, r : r + 1],
                imm2=0.1,
            )
            return v

        def emit_matmuls(half, cbase, tiles, r, v):
            c0 = max(cbase, r * 1024)
            w = cbase + HW - c0
            for s in range(w // MMF):
                gc = c0 + s * MMF
                top = min(gc // 1024, tiles - 1)
                if half == 0:
                    mm_start, mm_stop = (r == top), (r == 0)
                else:
                    mm_start, mm_stop = (r == 0), (r == top)
                nc.tensor.matmul(
                    Xh[half][:, gc - cbase : gc - cbase + MMF],
                    phib[:, r, :],
                    v[:, s * MMF : (s + 1) * MMF],
                    start=mm_start,
                    stop=mm_stop,
                )

        Xh[0] = pp.tile([K, HW], F32, tag="X", name="X0u")
        v7 = emit_main(0, HW, RT - 1)
        v6 = emit_main(0, HW, RT - 2)

        # ---- Phi[p, r, n] = sum_m A[m, n] * x_row^m (Horner), bf16 ----
        xrow = sp.tile([P, RT], F32)
        nc.vector.tensor_scalar_sub(xrow[:], urow_sb[:], 0.5)
        phit = sp.tile([P, RT, K], F32)
        nc.vector.tensor_copy(
            phit[:], abuf[:, DEG : DEG + 1, :].broadcast_to([P, RT, K])
        )
        xrow_b = xrow[:, :, None].broadcast_to([P, RT, K])
        for m in range(DEG - 1, -1, -1):
            nc.vector.tensor_mul(phit[:], phit[:], xrow_b)
            nc.vector.tensor_add(
                phit[:], phit[:], abuf[:, m : m + 1, :].broadcast_to([P, RT, K])
            )
        phib = constp.tile([P, RT, K], BF16)
        nc.vector.tensor_copy(phib[:], phit[:])

        # ---- Psi[n, j] = x_j^n via [P, 64] layout + DRAM bounce ----
        psi_dram = dramp.tile([K, N], F32)
        x64 = sp.tile([P, FB], F32)
        nc.vector.tensor_scalar_sub(x64[:], u64[:], 0.5)
        ones64 = sp.tile([P, FB], F32)
        nc.vector.memset(ones64[:], 1.0)
        nc.sync.dma_start(psi_dram[0, :].rearrange("(p f) -> p f", p=P), ones64[:])
        nc.sync.dma_start(psi_dram[1, :].rearrange("(p f) -> p f", p=P), x64[:])
        prev = x64
        for n in range(2, K):
            nxt = sp.tile([P, FB], F32, tag=f"pw{n}")
            nc.vector.tensor_mul(nxt[:], prev[:], x64[:])
            nc.sync.dma_start(psi_dram[n, :].rearrange("(p f) -> p f", p=P), nxt[:])
            prev = nxt
        # reshaped Psi for the overlapped drains; direct slices for the two
        # small trailing pieces
        psiR1 = constp.tile([K * 16, HW // 16], F32)
        psiR0a = constp.tile([K * 16, HW // 32], F32)
        for n in range(K):
            nc.sync.dma_start(
                psiR1[n * 16 : (n + 1) * 16, :],
                psi_dram[n : n + 1, HW:].rearrange("o (k f) -> o k f", k=16),
            )
            nc.sync.dma_start(
                psiR0a[n * 16 : (n + 1) * 16, :],
                psi_dram[n : n + 1, 0 : HW // 2].rearrange("o (k f) -> o k f", k=16),
            )
        psi0b = constp.tile([K, HW // 2], F32)
        nc.sync.dma_start(psi0b[:], psi_dram[:, HW // 2 : HW])

        accP = sp.tile([K * 16, 3], F32)
        nc.vector.memset(accP[:], 0.0)
        accA = accP[:, 0:1]
        accB = accP[:, 1:2]
        accC = accP[:K, 2:3]
        ttr_scr = sp.tile([K * 16, HW // 16], F32)
        ttr_scr2 = sp.tile([K, HW // 2], F32)



        emit_matmuls(0, HW, RT, RT - 1, v7)
        emit_matmuls(0, HW, RT, RT - 2, v6)
        for r in range(RT - 3, -1, -1):
            emit_group(0, HW, RT, r)
        Xsb1 = sp.tile([K, HW], F32)
        nc.scalar.copy(Xsb1[:], Xh[0][:])
        xr1 = sp.tile([K * 16, HW // 16], F32)
        for n in range(K):
            nc.sync.dma_start(
                xr1[n * 16 : (n + 1) * 16, :],
                Xsb1[n : n + 1, :].rearrange("o (k f) -> o k f", k=16),
            )
        Xh[1] = pp.tile([K, HW], F32, tag="X", name="X1l")
        for r in range(RT // 2):
            emit_group(1, 0, RT // 2, r)
        # drains: overlapped pieces go through ACT-copy + partition-reshape
        # DMA + a full-width mini-reduce; the tail piece reads PSUM directly.
        Xsb0a = sp.tile([K, HW // 2], F32)
        nc.scalar.copy(Xsb0a[:], Xh[1][:, 0 : HW // 2])
        xr0a = sp.tile([K * 16, HW // 32], F32)
        for n in range(K):
            nc.sync.dma_start(
                xr0a[n * 16 : (n + 1) * 16, :],
                Xsb0a[n : n + 1, :].rearrange("o (k f) -> o k f", k=16),
            )
        nc.vector._custom_dve(
            dve_ops.TENSOR_TENSOR_REDUCE,
            out=ttr_scr[:],
            in0=xr1[:],
            in1=psiR1[:],
            s0=0.0,
            s1=1.0,
            accum_out=accA,
        )
        nc.vector._custom_dve(
            dve_ops.TENSOR_TENSOR_REDUCE,
            out=ttr_scr[:, : HW // 32],
            in0=xr0a[:],
            in1=psiR0a[:],
            s0=0.0,
            s1=1.0,
            accum_out=accB,
        )
        nc.vector._custom_dve(
            dve_ops.TENSOR_TENSOR_REDUCE,
            out=ttr_scr2[:],
            in0=Xh[1][:, HW // 2 : HW],
            in1=psi0b[:],
            s0=0.0,
            s1=1.0,
            accum_out=accC,
        )
        nc.sync.dma_start(
            out_ext[0 : 3 * 112].rearrange("(p c) -> p c", c=3), accP[:]
        )

    nc.compile()
    return nc


_NC_CACHE = None


def _get_nc():
    global _NC_CACHE
    if _NC_CACHE is None:
        _NC_CACHE = _build_nc()
    return _NC_CACHE


def _exact_count(t: np.ndarray) -> int:
    n = t.shape[0]
    _, cnts = np.unique(t, return_counts=True)
    dup = int(sum(int(c) * (int(c) - 1) // 2 for c in cnts[cnts > 1]))
    return n * (n - 1) // 2 - dup


def _make_in_maps(predictions, targets, uncertainties):
    t = np.ascontiguousarray(np.asarray(targets, np.float32))
    p = np.ascontiguousarray(np.asarray(predictions, np.float32))
    u = np.ascontiguousarray(np.asarray(uncertainties, np.float32))
    # sort by target (loss is permutation invariant); stride rows across
    # cores so every core sees the same triangular-skip schedule.
    order = np.argsort(t, kind="stable")
    ts, ps, us = t[order], p[order], u[order]
    t01 = (np.float32(0.1) * ts).astype(np.float32)
    t01_h = t01.astype(np.float16)
    ps_h = ps.astype(np.float16)
    in_maps = []
    for i in range(NCORES):
        in_maps.append(
            {
                "t01col": t01_h,
                "pcol": ps_h,
                "ucol": us,
                "rows3": np.ascontiguousarray(
                    np.stack([t01[i::NCORES], ps[i::NCORES], us[i::NCORES]])
                ),
                "acoef": _ACOEF,
            }
        )
    return in_maps, t


def _run_device(in_maps, trace=False, **kw):
    nc = _get_nc()
    return run_bass_kernel_spmd(
        nc, in_maps, core_ids=list(range(NCORES)), trace=trace, **kw
    )


def kernel(predictions, targets, uncertainties):
    in_maps, t = _make_in_maps(predictions, targets, uncertainties)
    res = _run_device(in_maps)
    total = np.float64(0.0)
    for r in res.results:
        total += np.asarray(r["out"], np.float64).sum()
    count = _exact_count(t)
    return np.asarray(total / max(count, 1), dtype=np.float32)


# revision 31
# speedup vs baseline: 1.0241x; 1.0241x over previous
"""AdaptiveRankingLoss distributed Bass kernel for 8 TRN2 NeuronCores.

Math
----
reference loss = sum_{i<j, t_i != t_j} w_ij * relu(margin_ij - sign(t_i - t_j)*(p_i - p_j))
                 / count,
  margin = 0.1 * clip(|t_i - t_j|, 0.1, 1.0),  w = 1/(1 + u_i + u_j).

The summand is symmetric under i<->j, and splitting by the sign of
a = t_j - t_i gives an exactly equivalent full-matrix form with no sign(),
no abs() and no triangular mask:

    numerator = sum_{all i,j} [a_ij > 0] * w_ij * relu(clip(0.1*a_ij, .01, .1) - (p_j - p_i))

Ties (a == 0, including the diagonal) contribute exactly 0 via the
indicator, and `count` is computed exactly on the host from duplicate
analysis of t.

Device mapping (per core: 1024 rows x 8192 cols of the pair matrix)
------------------------------------------------------------------
* one custom 8-stage DVE op produces v = [a>0]*relu(clip(0.1a,.01,.1)-b)
  per element (fp32 internal, bf16 out), streaming the broadcast column
  vectors with the row values as per-partition scalars.
* the weight w = 1/(1+u_i+u_j) is applied through a degree-6 bilinear
  polynomial 1/(2+z) ~ p(z), z = x_i + x_j, x = u - 0.5:
      w_ij ~ sum_n Phi_n(x_i) * x_j^n
  so  sum_ij v_ij w_ij = sum_{n,j} X[n,j] * Psi[n,j]  with
      X[n,j] = sum_i Phi_n(x_i) v_ij   (TensorEngine matmul, PSUM accum)
      Psi[n,j] = x_j^n.
* drains: X is staged out of PSUM by the (otherwise idle) scalar engine,
  partition-reshaped to [112, 256] by DMA, and contracted against Psi by a
  fused custom multiply-reduce; the tail piece reads PSUM directly. The
  host sums the per-core accumulators and divides by the exact pair count.

Host-side marshalling: inputs are sorted by target (the loss is
permutation-invariant) with rows strided across cores, so the [a>0]
indicator becomes triangular and whole column ranges are provably zero
and skipped (bit-exact). Columns are pre-scaled by 0.1 and cast to fp16.
"""

import numpy as np

import concourse.bass as bass
import concourse.bacc as bacc
import concourse.mybir as mybir
import concourse.tile as tile
from concourse.bass_utils import run_bass_kernel_spmd
from concourse import dve_ops
from concourse.dve_spec import (
    Spec,
    Src0,
    Src1,
    C0,
    C1,
    C2,
    Zero,
    relu,
    maxx,
    minn,
    lower,
    _has_src1,
)
from concourse.dve_uop import DveOpSpec

F32 = mybir.dt.float32
BF16 = mybir.dt.bfloat16

N = 8192          # problem size (hardcoded per spec)
NCORES = 8
P = 128           # SBUF partitions
R = N // NCORES   # rows per core (1024)
RT = R // P       # row tiles per core (8)
FC = 1024         # column chunk
NCH = N // FC     # chunks (8)
DEG = 6           # weight polynomial degree
K = DEG + 1
MMF = 512         # matmul free-dim tile

# Inputs are sorted by target on the host and rows are strided across cores
# (core c gets sorted rows c, c+8, ...). Row-tile r of any core then covers
# sorted positions >= 1024*r, so column chunks c < r satisfy t_j <= t_i
# everywhere -> the [a>0] indicator is identically 0 and the chunk is skipped
# for that tile. Bit-exact with the unskipped computation.


# --------------------------------------------------------------------------
# custom DVE op: v = [Src0 - C0 > 0] * relu(clip(Src0 - C0, C2^2, C2) - (Src1 - C1))
# Src0 = 0.1*t_col, C0 = 0.1*t_row, Src1 = p_col, C1 = p_row, C2 = 0.1.
# --------------------------------------------------------------------------
_ARL_NAME = "ARL_MAIN_V1"


def _arl_reference(in0, in1, s0, s1, imm2):
    a = in0 - s0
    m = np.clip(a, np.float32(imm2) * np.float32(imm2), imm2)
    return (a > 0).astype(np.float32) * np.maximum(m - (in1 - s1), 0.0)


def _register_arl_op():
    for op in dve_ops.OPS:
        if op.name == _ARL_NAME:
            return op
    a = Src0 - C0
    m = minn(maxx(a, C2 * C2), C2)
    h = relu(m - (Src1 - C1))
    spec = Spec(body=(a > Zero) * h, reference=_arl_reference)
    row = dve_ops._CUSTOM_DVE_ROW_BASE + len(dve_ops.OPS)
    assert row < 0x20, "custom-DVE row overflow"
    dve_ops._SUB_OPCODE_FOR_NAME[_ARL_NAME] = row
    shas = {}
    for ver in ("v3", "v4"):
        try:
            uops = lower(spec, ver=ver)
            shas[ver] = DveOpSpec(
                name=_ARL_NAME, opcode=row, uops=uops, rd1_en=_has_src1(spec)
            ).sha(ver)
        except Exception:
            pass
    op = dve_ops.DveOp(_ARL_NAME, spec, subdim=False, uops_sha=shas)
    dve_ops.OPS.append(op)
    dve_ops.CUSTOM_DVE_SPECS[_ARL_NAME] = spec
    return op


ARL_MAIN = _register_arl_op()


# --------------------------------------------------------------------------
# degree-6 bilinear split of w = 1/(1+u_i+u_j) = 1/(2 + x_i + x_j), x = u-.5
# --------------------------------------------------------------------------
def _acoef_matrix() -> np.ndarray:
    from numpy.polynomial import chebyshev as _C
    from math import comb

    nodes = np.cos((2 * np.arange(DEG + 1) + 1) / (2 * (DEG + 1)) * np.pi)
    ch = _C.chebfit(nodes, 1.0 / (2.0 + nodes), DEG)
    c = _C.cheb2poly(ch)  # power-basis coeffs of p(z) ~ 1/(2+z) on [-1,1]
    A = np.zeros((K, K), np.float64)
    for mm in range(K):
        for nn in range(K):
            if mm + nn <= DEG:
                A[mm, nn] = c[mm + nn] * comb(mm + nn, mm)
    return A.astype(np.float32)


_ACOEF = _acoef_matrix()


# --------------------------------------------------------------------------
# device graph builder
# --------------------------------------------------------------------------
def _build_nc():
    from contextlib import ExitStack

    F16 = mybir.dt.float16
    HW = N // 2  # column half-width (4096)

    nc = bacc.Bacc(None, target_bir_lowering=False, debug=False)

    t01_ext = nc.declare_dram_parameter("t01col", [N], F16, isOutput=False)
    p_ext = nc.declare_dram_parameter("pcol", [N], F16, isOutput=False)
    u_ext = nc.declare_dram_parameter("ucol", [N], F32, isOutput=False)
    rows_ext = nc.declare_dram_parameter("rows3", [3, R], F32, isOutput=False)
    a_ext = nc.declare_dram_parameter("acoef", [K, K], F32, isOutput=False)
    out_ext = nc.declare_dram_parameter("out", [448], F32, isOutput=True)

    with tile.TileContext(nc) as tc, ExitStack() as ctx:
        constp = ctx.enter_context(tc.tile_pool(name="const", bufs=1))
        colp = ctx.enter_context(tc.tile_pool(name="cols", bufs=1))
        vp = ctx.enter_context(tc.tile_pool(name="v", bufs=3))
        pp = ctx.enter_context(tc.tile_pool(name="psum", bufs=1, space="PSUM"))
        sp = ctx.enter_context(tc.tile_pool(name="small", bufs=1))
        dramp = ctx.enter_context(tc.tile_pool(name="dram", bufs=1, space="DRAM"))

        # ---- small prep DMAs; one fused row-scalar load + u/a coefs ----
        rows_sb = constp.tile([P, 3, RT], F32)
        rows_src = bass.AP(
            tensor=rows_ext, offset=0, ap=[[1, P], [R, 3], [P, RT]]
        )
        nc.sync.dma_start(rows_sb[:], rows_src)
        t01row_sb = rows_sb[:, 0, :]
        prow_sb = rows_sb[:, 1, :]
        urow_sb = rows_sb[:, 2, :]
        # ---- full-width fp16 column tiles; ranges in processing order,
        # small prep loads slotted after the first two ----
        t01_sb = colp.tile([P, N], F16)
        p_sb = colp.tile([P, N], F16)

        def load_cols(lo, w):
            nc.sync.dma_start(
                t01_sb[:, lo : lo + w],
                bass.AP(tensor=t01_ext, offset=lo, ap=[[0, P], [1, w]]),
            )
            nc.sync.dma_start(
                p_sb[:, lo : lo + w],
                bass.AP(tensor=p_ext, offset=lo, ap=[[0, P], [1, w]]),
            )

        load_cols(7168, 1024)
        load_cols(6144, 1024)
        abuf = constp.tile([P, K, K], F32)
        a_src = bass.AP(tensor=a_ext, offset=0, ap=[[0, P], [K, K], [1, K]])
        nc.sync.dma_start(abuf[:], a_src)
        FB = N // P  # 64
        u64 = sp.tile([P, FB], F32)
        nc.sync.dma_start(u64[:], u_ext[:].rearrange("(p f) -> p f", p=P))
        load_cols(5120, 1024)
        load_cols(4096, 1024)
        load_cols(2048, 2048)
        load_cols(0, 2048)

        # ---- pairwise compute (see module docstring). The two smallest
        # upper-half row-tiles are emitted before the Phi/Psi prep so the
        # DVE starts the bulk work as soon as the tail columns land; prep
        # then fills the stream while the remaining columns load. ----
        Xh = {}

        def emit_main(half, cbase, r):
            c0 = max(cbase, r * 1024)
            w = cbase + HW - c0
            v = vp.tile([P, HW], BF16, tag="v", name=f"v{half}_{r}")
            nc.vector._custom_dve(
                ARL_MAIN,
                out=v[:, :w],
                in0=t01_sb[:, c0 : cbase + HW],
                in1=p_sb[:, c0 : cbase + HW],
                s0=t01row_sb[:, r : r + 1],
                s1=prow_sb[:, r : r + 1],
                imm2=0.1,
            )
            return v

        def emit_matmuls(half, cbase, tiles, r, v):
            c0 = max(cbase, r * 1024)
            w = cbase + HW - c0
            for s in range(w // MMF):
                gc = c0 + s * MMF
                top = min(gc // 1024, tiles - 1)
                if half == 0:
                    mm_start, mm_stop = (r == top), (r == 0)
                else:
                    mm_start, mm_stop = (r == 0), (r == top)
                nc.tensor.matmul(
                    Xh[half][:, gc - cbase : gc - cbase + MMF],
                    phib[:, r, :],
                    v[:, s * MMF : (s + 1) * MMF],
                    start=mm_start,
                    stop=mm_stop,
                )

        def emit_group(half, cbase, tiles, r):
            v = emit_main(half, cbase, r)
            emit_matmuls(half, cbase, tiles, r, v)

        Xh[0] = pp.tile([K, HW], F32, tag="X", name="X0u")
        v7 = emit_main(0, HW, RT - 1)
        v6 = emit_main(0, HW, RT - 2)

        # ---- Phi[p, r, n] = sum_m A[m, n] * x_row^m (Horner), bf16 ----
        xrow = sp.tile([P, RT], F32)
        nc.vector.tensor_scalar_sub(xrow[:], urow_sb[:], 0.5)
        phit = sp.tile([P, RT, K], F32)
        nc.vector.tensor_copy(
            phit[:], abuf[:, DEG : DEG + 1, :].broadcast_to([P, RT, K])
        )
        xrow_b = xrow[:, :, None].broadcast_to([P, RT, K])
        for m in range(DEG - 1, -1, -1):
            nc.vector.tensor_mul(phit[:], phit[:], xrow_b)
            nc.vector.tensor_add(
                phit[:], phit[:], abuf[:, m : m + 1, :].broadcast_to([P, RT, K])
            )
        phib = constp.tile([P, RT, K], BF16)
        nc.vector.tensor_copy(phib[:], phit[:])

        # ---- Psi[n, j] = x_j^n via [P, 64] layout + DRAM bounce ----
        psi_dram = dramp.tile([K, N], F32)
        x64 = sp.tile([P, FB], F32)
        nc.vector.tensor_scalar_sub(x64[:], u64[:], 0.5)
        ones64 = sp.tile([P, FB], F32)
        nc.vector.memset(ones64[:], 1.0)
        nc.sync.dma_start(psi_dram[0, :].rearrange("(p f) -> p f", p=P), ones64[:])
        nc.sync.dma_start(psi_dram[1, :].rearrange("(p f) -> p f", p=P), x64[:])
        prev = x64
        for n in range(2, K):
            nxt = sp.tile([P, FB], F32, tag=f"pw{n}")
            nc.vector.tensor_mul(nxt[:], prev[:], x64[:])
            nc.sync.dma_start(psi_dram[n, :].rearrange("(p f) -> p f", p=P), nxt[:])
            prev = nxt
        # reshaped Psi for the overlapped drains; direct slices for the two
        # small trailing pieces
        psiR1 = constp.tile([K * 16, HW // 16], F32)
        psiR0a = constp.tile([K * 16, HW // 32], F32)
        for n in range(K):
            nc.sync.dma_start(
                psiR1[n * 16 : (n + 1) * 16, :],
                psi_dram[n : n + 1, HW:].rearrange("o (k f) -> o k f", k=16),
            )
            nc.sync.dma_start(
                psiR0a[n * 16 : (n + 1) * 16, :],
                psi_dram[n : n + 1, 0 : HW // 2].rearrange("o (k f) -> o k f", k=16),
            )
        psi0b = constp.tile([K, HW // 2], F32)
        nc.sync.dma_start(psi0b[:], psi_dram[:, HW // 2 : HW])

        accP = sp.tile([K * 16, 4], F32)
        nc.vector.memset(accP[:], 0.0)
        accA = accP[:, 0:1]
        accB = accP[:, 1:2]
        accC = accP[:K, 2:3]
        accD = accP[:K, 3:4]
        ttr_scr = sp.tile([K * 16, HW // 16], F32)
        ttr_scr2 = sp.tile([K, HW // 2], F32)



        emit_matmuls(0, HW, RT, RT - 1, v7)
        emit_matmuls(0, HW, RT, RT - 2, v6)
        for r in range(RT - 3, -1, -1):
            emit_group(0, HW, RT, r)
        Xsb1 = sp.tile([K, HW], F32)
        nc.scalar.copy(Xsb1[:], Xh[0][:])
        xr1 = sp.tile([K * 16, HW // 16], F32)
        for n in range(K):
            nc.sync.dma_start(
                xr1[n * 16 : (n + 1) * 16, :],
                Xsb1[n : n + 1, :].rearrange("o (k f) -> o k f", k=16),
            )
        Xh[1] = pp.tile([K, HW], F32, tag="X", name="X1l")
        for r in range(RT // 2):
            emit_group(1, 0, RT // 2, r)
        # drains: overlapped pieces go through ACT-copy + partition-reshape
        # DMA + a full-width mini-reduce; the tail piece reads PSUM directly.
        Xsb0a = sp.tile([K, HW // 2], F32)
        nc.scalar.copy(Xsb0a[:], Xh[1][:, 0 : HW // 2])
        xr0a = sp.tile([K * 16, HW // 32], F32)
        for n in range(K):
            nc.sync.dma_start(
                xr0a[n * 16 : (n + 1) * 16, :],
                Xsb0a[n : n + 1, :].rearrange("o (k f) -> o k f", k=16),
            )
        nc.vector._custom_dve(
            dve_ops.TENSOR_TENSOR_REDUCE,
            out=ttr_scr[:],
            in0=xr1[:],
            in1=psiR1[:],
            s0=0.0,
            s1=1.0,
            accum_out=accA,
        )
        nc.vector._custom_dve(
            dve_ops.TENSOR_TENSOR_REDUCE,
            out=ttr_scr[:, : HW // 32],
            in0=xr0a[:],
            in1=psiR0a[:],
            s0=0.0,
            s1=1.0,
            accum_out=accB,
        )
        nc.vector._custom_dve(
            dve_ops.TENSOR_TENSOR_REDUCE,
            out=ttr_scr2[:, :1024],
            in0=Xh[1][:, HW // 2 : HW // 2 + 1024],
            in1=psi0b[:, :1024],
            s0=0.0,
            s1=1.0,
            accum_out=accC,
        )
        nc.vector._custom_dve(
            dve_ops.TENSOR_TENSOR_REDUCE,
            out=ttr_scr2[:, :1024],
            in0=Xh[1][:, HW // 2 + 1024 : HW],
            in1=psi0b[:, 1024:],
            s0=0.0,
            s1=1.0,
            accum_out=accD,
        )
        nc.sync.dma_start(
            out_ext[0 : 4 * 112].rearrange("(p c) -> p c", c=4), accP[:]
        )

    nc.compile()
    return nc


_NC_CACHE = None


def _get_nc():
    global _NC_CACHE
    if _NC_CACHE is None:
        _NC_CACHE = _build_nc()
    return _NC_CACHE


def _exact_count(t: np.ndarray) -> int:
    n = t.shape[0]
    _, cnts = np.unique(t, return_counts=True)
    dup = int(sum(int(c) * (int(c) - 1) // 2 for c in cnts[cnts > 1]))
    return n * (n - 1) // 2 - dup


def _make_in_maps(predictions, targets, uncertainties):
    t = np.ascontiguousarray(np.asarray(targets, np.float32))
    p = np.ascontiguousarray(np.asarray(predictions, np.float32))
    u = np.ascontiguousarray(np.asarray(uncertainties, np.float32))
    # sort by target (loss is permutation invariant); stride rows across
    # cores so every core sees the same triangular-skip schedule.
    order = np.argsort(t, kind="stable")
    ts, ps, us = t[order], p[order], u[order]
    t01 = (np.float32(0.1) * ts).astype(np.float32)
    t01_h = t01.astype(np.float16)
    ps_h = ps.astype(np.float16)
    in_maps = []
    for i in range(NCORES):
        in_maps.append(
            {
                "t01col": t01_h,
                "pcol": ps_h,
                "ucol": us,
                "rows3": np.ascontiguousarray(
                    np.stack([t01[i::NCORES], ps[i::NCORES], us[i::NCORES]])
                ),
                "acoef": _ACOEF,
            }
        )
    return in_maps, t


def _run_device(in_maps, trace=False, **kw):
    nc = _get_nc()
    return run_bass_kernel_spmd(
        nc, in_maps, core_ids=list(range(NCORES)), trace=trace, **kw
    )


def kernel(predictions, targets, uncertainties):
    in_maps, t = _make_in_maps(predictions, targets, uncertainties)
    res = _run_device(in_maps)
    total = np.float64(0.0)
    for r in res.results:
        total += np.asarray(r["out"], np.float64).sum()
    count = _exact_count(t)
    return np.asarray(total / max(count, 1), dtype=np.float32)


# revision 34
# speedup vs baseline: 1.0298x; 1.0056x over previous
"""AdaptiveRankingLoss distributed Bass kernel for 8 TRN2 NeuronCores.

Math
----
reference loss = sum_{i<j, t_i != t_j} w_ij * relu(margin_ij - sign(t_i - t_j)*(p_i - p_j))
                 / count,
  margin = 0.1 * clip(|t_i - t_j|, 0.1, 1.0),  w = 1/(1 + u_i + u_j).

The summand is symmetric under i<->j, and splitting by the sign of
a = t_j - t_i gives an exactly equivalent full-matrix form with no sign(),
no abs() and no triangular mask:

    numerator = sum_{all i,j} [a_ij > 0] * w_ij * relu(clip(0.1*a_ij, .01, .1) - (p_j - p_i))

Ties (a == 0, including the diagonal) contribute exactly 0 via the
indicator, and `count` is computed exactly on the host from duplicate
analysis of t.

Device mapping (per core: 1024 rows x 8192 cols of the pair matrix)
------------------------------------------------------------------
* one custom 8-stage DVE op produces v = [a>0]*relu(clip(0.1a,.01,.1)-b)
  per element (fp32 internal, bf16 out), streaming the broadcast column
  vectors with the row values as per-partition scalars.
* the weight w = 1/(1+u_i+u_j) is applied through a degree-6 bilinear
  polynomial 1/(2+z) ~ p(z), z = x_i + x_j, x = u - 0.5:
      w_ij ~ sum_n Phi_n(x_i) * x_j^n
  so  sum_ij v_ij w_ij = sum_{n,j} X[n,j] * Psi[n,j]  with
      X[n,j] = sum_i Phi_n(x_i) v_ij   (TensorEngine matmul, PSUM accum)
      Psi[n,j] = x_j^n.
* drains: X is staged out of PSUM by the (otherwise idle) scalar engine,
  partition-reshaped to [112, 256] by DMA, and contracted against Psi by a
  fused custom multiply-reduce; the tail piece reads PSUM directly. The
  host sums the per-core accumulators and divides by the exact pair count.

Host-side marshalling: inputs are sorted by target (the loss is
permutation-invariant) with rows strided across cores, so the [a>0]
indicator becomes triangular and whole column ranges are provably zero
and skipped (bit-exact). Columns are pre-scaled by 0.1 and cast to fp16.
"""

import numpy as np

import concourse.bass as bass
import concourse.bacc as bacc
import concourse.mybir as mybir
import concourse.tile as tile
from concourse.bass_utils import run_bass_kernel_spmd
from concourse import dve_ops
from concourse.dve_spec import (
    Spec,
    Src0,
    Src1,
    C0,
    C1,
    C2,
    Zero,
    relu,
    maxx,
    minn,
    lower,
    _has_src1,
)
from concourse.dve_uop import DveOpSpec

F32 = mybir.dt.float32
BF16 = mybir.dt.bfloat16

N = 8192          # problem size (hardcoded per spec)
NCORES = 8
P = 128           # SBUF partitions
R = N // NCORES   # rows per core (1024)
RT = R // P       # row tiles per core (8)
FC = 1024         # column chunk
NCH = N // FC     # chunks (8)
DEG = 6           # weight polynomial degree
K = DEG + 1
MMF = 512         # matmul free-dim tile

# Inputs are sorted by target on the host and rows are strided across cores
# (core c gets sorted rows c, c+8, ...). Row-tile r of any core then covers
# sorted positions >= 1024*r, so column chunks c < r satisfy t_j <= t_i
# everywhere -> the [a>0] indicator is identically 0 and the chunk is skipped
# for that tile. Bit-exact with the unskipped computation.


# --------------------------------------------------------------------------
# custom DVE op: v = [Src0 - C0 > 0] * relu(clip(Src0 - C0, C2^2, C2) - (Src1 - C1))
# Src0 = 0.1*t_col, C0 = 0.1*t_row, Src1 = p_col, C1 = p_row, C2 = 0.1.
# --------------------------------------------------------------------------
_ARL_NAME = "ARL_MAIN_V1"


def _arl_reference(in0, in1, s0, s1, imm2):
    a = in0 - s0
    m = np.clip(a, np.float32(imm2) * np.float32(imm2), imm2)
    return (a > 0).astype(np.float32) * np.maximum(m - (in1 - s1), 0.0)


def _register_arl_op():
    for op in dve_ops.OPS:
        if op.name == _ARL_NAME:
            return op
    a = Src0 - C0
    m = minn(maxx(a, C2 * C2), C2)
    h = relu(m - (Src1 - C1))
    spec = Spec(body=(a > Zero) * h, reference=_arl_reference)
    row = dve_ops._CUSTOM_DVE_ROW_BASE + len(dve_ops.OPS)
    assert row < 0x20, "custom-DVE row overflow"
    dve_ops._SUB_OPCODE_FOR_NAME[_ARL_NAME] = row
    shas = {}
    for ver in ("v3", "v4"):
        try:
            uops = lower(spec, ver=ver)
            shas[ver] = DveOpSpec(
                name=_ARL_NAME, opcode=row, uops=uops, rd1_en=_has_src1(spec)
            ).sha(ver)
        except Exception:
            pass
    op = dve_ops.DveOp(_ARL_NAME, spec, subdim=False, uops_sha=shas)
    dve_ops.OPS.append(op)
    dve_ops.CUSTOM_DVE_SPECS[_ARL_NAME] = spec
    return op


ARL_MAIN = _register_arl_op()


# --------------------------------------------------------------------------
# degree-6 bilinear split of w = 1/(1+u_i+u_j) = 1/(2 + x_i + x_j), x = u-.5
# --------------------------------------------------------------------------
def _acoef_matrix() -> np.ndarray:
    from numpy.polynomial import chebyshev as _C
    from math import comb

    nodes = np.cos((2 * np.arange(DEG + 1) + 1) / (2 * (DEG + 1)) * np.pi)
    ch = _C.chebfit(nodes, 1.0 / (2.0 + nodes), DEG)
    c = _C.cheb2poly(ch)  # power-basis coeffs of p(z) ~ 1/(2+z) on [-1,1]
    A = np.zeros((K, K), np.float64)
    for mm in range(K):
        for nn in range(K):
            if mm + nn <= DEG:
                A[mm, nn] = c[mm + nn] * comb(mm + nn, mm)
    return A.astype(np.float32)


_ACOEF = _acoef_matrix()


# --------------------------------------------------------------------------
# device graph builder
# --------------------------------------------------------------------------
def _build_nc():
    from contextlib import ExitStack

    F16 = mybir.dt.float16
    HW = N // 2  # column half-width (4096)

    nc = bacc.Bacc(None, target_bir_lowering=False, debug=False)

    t01_ext = nc.declare_dram_parameter("t01col", [N], F16, isOutput=False)
    p_ext = nc.declare_dram_parameter("pcol", [N], F16, isOutput=False)
    u_ext = nc.declare_dram_parameter("ucol", [N], F32, isOutput=False)
    rows_ext = nc.declare_dram_parameter("rows3", [P, 3 * RT], F32, isOutput=False)
    a_ext = nc.declare_dram_parameter("acoef", [K, K], F32, isOutput=False)
    out_ext = nc.declare_dram_parameter("out", [448], F32, isOutput=True)

    with tile.TileContext(nc) as tc, ExitStack() as ctx:
        constp = ctx.enter_context(tc.tile_pool(name="const", bufs=1))
        colp = ctx.enter_context(tc.tile_pool(name="cols", bufs=1))
        vp = ctx.enter_context(tc.tile_pool(name="v", bufs=3))
        pp = ctx.enter_context(tc.tile_pool(name="psum", bufs=1, space="PSUM"))
        sp = ctx.enter_context(tc.tile_pool(name="small", bufs=1))
        dramp = ctx.enter_context(tc.tile_pool(name="dram", bufs=1, space="DRAM"))

        # ---- small prep DMAs; one fused row-scalar load + u/a coefs ----
        rows_sb = constp.tile([P, 3, RT], F32)
        nc.sync.dma_start(
            rows_sb[:], rows_ext[:, :].rearrange("p (s r) -> p s r", s=3)
        )
        t01row_sb = rows_sb[:, 0, :]
        prow_sb = rows_sb[:, 1, :]
        urow_sb = rows_sb[:, 2, :]
        # ---- full-width fp16 column tiles; ranges in processing order,
        # small prep loads slotted after the first two ----
        t01_sb = colp.tile([P, N], F16)
        p_sb = colp.tile([P, N], F16)

        def load_cols(lo, w):
            nc.sync.dma_start(
                t01_sb[:, lo : lo + w],
                bass.AP(tensor=t01_ext, offset=lo, ap=[[0, P], [1, w]]),
            )
            nc.sync.dma_start(
                p_sb[:, lo : lo + w],
                bass.AP(tensor=p_ext, offset=lo, ap=[[0, P], [1, w]]),
            )

        load_cols(7168, 1024)
        load_cols(6144, 1024)
        abuf = constp.tile([P, K, K], F32)
        a_src = bass.AP(tensor=a_ext, offset=0, ap=[[0, P], [K, K], [1, K]])
        nc.sync.dma_start(abuf[:], a_src)
        FB = N // P  # 64
        u64 = sp.tile([P, FB], F32)
        nc.sync.dma_start(u64[:], u_ext[:].rearrange("(p f) -> p f", p=P))
        load_cols(5120, 1024)
        load_cols(4096, 1024)
        load_cols(2048, 2048)
        load_cols(0, 2048)

        # ---- pairwise compute (see module docstring). The two smallest
        # upper-half row-tiles are emitted before the Phi/Psi prep so the
        # DVE starts the bulk work as soon as the tail columns land; prep
        # then fills the stream while the remaining columns load. ----
        Xh = {}

        def emit_main(half, cbase, r):
            c0 = max(cbase, r * 1024)
            w = cbase + HW - c0
            v = vp.tile([P, HW], BF16, tag="v", name=f"v{half}_{r}")
            nc.vector._custom_dve(
                ARL_MAIN,
                out=v[:, :w],
                in0=t01_sb[:, c0 : cbase + HW],
                in1=p_sb[:, c0 : cbase + HW],
                s0=t01row_sb[:, r : r + 1],
                s1=prow_sb[:, r : r + 1],
                imm2=0.1,
            )
            return v

        def emit_matmuls(half, cbase, tiles, r, v):
            c0 = max(cbase, r * 1024)
            w = cbase + HW - c0
            for s in range(w // MMF):
                gc = c0 + s * MMF
                top = min(gc // 1024, tiles - 1)
                if half == 0:
                    mm_start, mm_stop = (r == top), (r == 0)
                else:
                    mm_start, mm_stop = (r == 0), (r == top)
                nc.tensor.matmul(
                    Xh[half][:, gc - cbase : gc - cbase + MMF],
                    phib[:, r, :],
                    v[:, s * MMF : (s + 1) * MMF],
                    start=mm_start,
                    stop=mm_stop,
                )

        def emit_group(half, cbase, tiles, r):
            v = emit_main(half, cbase, r)
            emit_matmuls(half, cbase, tiles, r, v)

        Xh[0] = pp.tile([K, HW], F32, tag="X", name="X0u")
        v7 = emit_main(0, HW, RT - 1)
        v6 = emit_main(0, HW, RT - 2)

        # ---- Phi[p, r, n] = sum_m A[m, n] * x_row^m (Horner), bf16 ----
        xrow = sp.tile([P, RT], F32)
        nc.vector.tensor_scalar_sub(xrow[:], urow_sb[:], 0.5)
        phit = sp.tile([P, RT, K], F32)
        nc.vector.tensor_copy(
            phit[:], abuf[:, DEG : DEG + 1, :].broadcast_to([P, RT, K])
        )
        xrow_b = xrow[:, :, None].broadcast_to([P, RT, K])
        for m in range(DEG - 1, -1, -1):
            nc.vector.tensor_mul(phit[:], phit[:], xrow_b)
            nc.vector.tensor_add(
                phit[:], phit[:], abuf[:, m : m + 1, :].broadcast_to([P, RT, K])
            )
        phib = constp.tile([P, RT, K], BF16)
        nc.vector.tensor_copy(phib[:], phit[:])

        # ---- Psi[n, j] = x_j^n via [P, 64] layout + DRAM bounce ----
        psi_dram = dramp.tile([K, N], F32)
        x64 = sp.tile([P, FB], F32)
        nc.vector.tensor_scalar_sub(x64[:], u64[:], 0.5)
        ones64 = sp.tile([P, FB], F32)
        nc.vector.memset(ones64[:], 1.0)
        nc.sync.dma_start(psi_dram[0, :].rearrange("(p f) -> p f", p=P), ones64[:])
        nc.sync.dma_start(psi_dram[1, :].rearrange("(p f) -> p f", p=P), x64[:])
        prev = x64
        for n in range(2, K):
            nxt = sp.tile([P, FB], F32, tag=f"pw{n}")
            nc.vector.tensor_mul(nxt[:], prev[:], x64[:])
            nc.sync.dma_start(psi_dram[n, :].rearrange("(p f) -> p f", p=P), nxt[:])
            prev = nxt
        # reshaped Psi for the overlapped drains; direct slices for the two
        # small trailing pieces
        psiR1 = constp.tile([K * 16, HW // 16], F32)
        psiR0a = constp.tile([K * 16, HW // 32], F32)
        for n in range(K):
            nc.sync.dma_start(
                psiR1[n * 16 : (n + 1) * 16, :],
                psi_dram[n : n + 1, HW:].rearrange("o (k f) -> o k f", k=16),
            )
            nc.sync.dma_start(
                psiR0a[n * 16 : (n + 1) * 16, :],
                psi_dram[n : n + 1, 0 : HW // 2].rearrange("o (k f) -> o k f", k=16),
            )
        psi0b = constp.tile([K, HW // 2], F32)
        nc.sync.dma_start(psi0b[:], psi_dram[:, HW // 2 : HW])

        accP = sp.tile([K * 16, 4], F32)
        nc.vector.memset(accP[:], 0.0)
        accA = accP[:, 0:1]
        accB = accP[:, 1:2]
        accC = accP[:K, 2:3]
        accD = accP[:K, 3:4]
        ttr_scr = sp.tile([K * 16, HW // 16], F32)
        ttr_scr2 = sp.tile([K, HW // 2], F32)



        emit_matmuls(0, HW, RT, RT - 1, v7)
        emit_matmuls(0, HW, RT, RT - 2, v6)
        for r in range(RT - 3, -1, -1):
            emit_group(0, HW, RT, r)
        Xsb1 = sp.tile([K, HW], F32)
        nc.scalar.copy(Xsb1[:], Xh[0][:])
        xr1 = sp.tile([K * 16, HW // 16], F32)
        for n in range(K):
            nc.sync.dma_start(
                xr1[n * 16 : (n + 1) * 16, :],
                Xsb1[n : n + 1, :].rearrange("o (k f) -> o k f", k=16),
            )
        Xh[1] = pp.tile([K, HW], F32, tag="X", name="X1l")
        for r in range(RT // 2):
            emit_group(1, 0, RT // 2, r)
        # drains: overlapped pieces go through ACT-copy + partition-reshape
        # DMA + a full-width mini-reduce; the tail piece reads PSUM directly.
        Xsb0a = sp.tile([K, HW // 2], F32)
        nc.scalar.copy(Xsb0a[:], Xh[1][:, 0 : HW // 2])
        xr0a = sp.tile([K * 16, HW // 32], F32)
        for n in range(K):
            nc.sync.dma_start(
                xr0a[n * 16 : (n + 1) * 16, :],
                Xsb0a[n : n + 1, :].rearrange("o (k f) -> o k f", k=16),
            )
        nc.vector._custom_dve(
            dve_ops.TENSOR_TENSOR_REDUCE,
            out=ttr_scr[:],
            in0=xr1[:],
            in1=psiR1[:],
            s0=0.0,
            s1=1.0,
            accum_out=accA,
        )
        nc.vector._custom_dve(
            dve_ops.TENSOR_TENSOR_REDUCE,
            out=ttr_scr[:, : HW // 32],
            in0=xr0a[:],
            in1=psiR0a[:],
            s0=0.0,
            s1=1.0,
            accum_out=accB,
        )
        nc.vector._custom_dve(
            dve_ops.TENSOR_TENSOR_REDUCE,
            out=ttr_scr2[:, :1024],
            in0=Xh[1][:, HW // 2 : HW // 2 + 1024],
            in1=psi0b[:, :1024],
            s0=0.0,
            s1=1.0,
            accum_out=accC,
        )
        nc.vector._custom_dve(
            dve_ops.TENSOR_TENSOR_REDUCE,
            out=ttr_scr2[:, :1024],
            in0=Xh[1][:, HW // 2 + 1024 : HW],
            in1=psi0b[:, 1024:],
            s0=0.0,
            s1=1.0,
            accum_out=accD,
        )
        nc.sync.dma_start(
            out_ext[0 : 4 * 112].rearrange("(p c) -> p c", c=4), accP[:]
        )

    nc.compile()
    return nc


_NC_CACHE = None


def _get_nc():
    global _NC_CACHE
    if _NC_CACHE is None:
        _NC_CACHE = _build_nc()
    return _NC_CACHE


def _exact_count(t: np.ndarray) -> int:
    n = t.shape[0]
    _, cnts = np.unique(t, return_counts=True)
    dup = int(sum(int(c) * (int(c) - 1) // 2 for c in cnts[cnts > 1]))
    return n * (n - 1) // 2 - dup


def _make_in_maps(predictions, targets, uncertainties):
    t = np.ascontiguousarray(np.asarray(targets, np.float32))
    p = np.ascontiguousarray(np.asarray(predictions, np.float32))
    u = np.ascontiguousarray(np.asarray(uncertainties, np.float32))
    # sort by target (loss is permutation invariant); stride rows across
    # cores so every core sees the same triangular-skip schedule.
    order = np.argsort(t, kind="stable")
    ts, ps, us = t[order], p[order], u[order]
    t01 = (np.float32(0.1) * ts).astype(np.float32)
    t01_h = t01.astype(np.float16)
    ps_h = ps.astype(np.float16)
    in_maps = []
    for i in range(NCORES):
        in_maps.append(
            {
                "t01col": t01_h,
                "pcol": ps_h,
                "ucol": us,
                "rows3": np.ascontiguousarray(
                    np.stack([t01[i::NCORES], ps[i::NCORES], us[i::NCORES]])
                    .reshape(3, RT, P)
                    .transpose(2, 0, 1)
                    .reshape(P, 3 * RT)
                ),
                "acoef": _ACOEF,
            }
        )
    return in_maps, t


def _run_device(in_maps, trace=False, **kw):
    nc = _get_nc()
    return run_bass_kernel_spmd(
        nc, in_maps, core_ids=list(range(NCORES)), trace=trace, **kw
    )


def kernel(predictions, targets, uncertainties):
    in_maps, t = _make_in_maps(predictions, targets, uncertainties)
    res = _run_device(in_maps)
    total = np.float64(0.0)
    for r in res.results:
        total += np.asarray(r["out"], np.float64).sum()
    count = _exact_count(t)
    return np.asarray(total / max(count, 1), dtype=np.float32)
